# revision 8
# baseline (speedup 1.0000x reference)
"""AdaptiveVectorModifier Trainium2 kernel (8 NeuronCores, data-parallel over rows).

Reference computation (per row n of x flattened to (N=8192, V=2048)):
    feats = x @ W_map.T                  (N, 128)
    h     = silu(feats @ W1.T + b1)      (N, 512)
    A     = (h @ W2.T + b2)              (N, 128, 128)
    feats2= einsum('nij,nj->ni', A, feats)
    out   = x + feats2 @ W_map

Sharding: rows split 8 ways (1024 rows/core); weights replicated.

Everything on-chip is computed in "transposed space" (rows on the free dim)
so that every matmul contraction dim lands on SBUF partitions:
    s1: featsT (m,n)  = sum_v W_mapT[v,m] xT[v,n]           f32r matmuls
    s2: hT     (k,n)  = silu(sum_m W1T[m,k] featsT[m,n]+b1) bf16
    s3: A_t    (j,n)  = sum_k W2T[k, 128t+j] hT[k,n]        bf16 (t = i index)
    s4: P_t    (j,n)  = (A_t + b2[128t+j]) * featsT[j,n]    DVE fused
        feats2_nat[n,t] = sum_j P_t[j,n]                    N=1 matmuls vs ones
    s5: modT   (v,n)  = sum_i W_map[i,v] feats2T[i,n];  outT = modT + xT
Host transposes x per shard on the way in and out.T on the way back.
"""

import numpy as np
import ml_dtypes

import concourse.bass as bass
import concourse.mybir as mybir
import concourse.tile as tile
from concourse import bacc
from concourse.masks import make_identity

F32 = mybir.dt.float32
F32R = mybir.dt.float32r
BF16 = mybir.dt.bfloat16
AF = mybir.ActivationFunctionType
ALU = mybir.AluOpType

V = 2048     # vector dim
M = 128      # mod dim
K = 512      # hidden (4*M)
NL = 1024    # rows per core
NB = 512     # rows per block
N_CORES = 8
SKEW = 2     # s3 -> reduce software-pipeline skew (t-loop)


def build_graph(n_rows=NL, silu_via_sigmoid=False):
    assert n_rows % NB == 0
    nblk = n_rows // NB

    nc = bacc.Bacc(None, target_bir_lowering=False)

    xT_d = nc.declare_dram_parameter("xT", [V, n_rows], F32, isOutput=False)
    xTb_d = nc.declare_dram_parameter("xTb", [V, n_rows], BF16, isOutput=False)
    w_mapT_d = nc.declare_dram_parameter("w_mapT", [V, M], BF16, isOutput=False)
    w_map_d = nc.declare_dram_parameter("w_map", [M, V], BF16, isOutput=False)
    w1T_d = nc.declare_dram_parameter("w1T", [M, K], BF16, isOutput=False)
    b1c_d = nc.declare_dram_parameter("b1c", [M, K // M], F32, isOutput=False)
    w2T_d = nc.declare_dram_parameter("w2T", [K, M * M], BF16, isOutput=False)
    b2r_d = nc.declare_dram_parameter("b2r", [M, M], F32, isOutput=False)
    out_d = nc.declare_dram_parameter("out", [V, n_rows], F32, isOutput=True)

    VC = V // M            # 16 chunks of the vector dim
    KC = K // M            # 4 chunks of the hidden dim
    NCH = NB // M          # 4 row-chunks per block (for the s4 reduce)

    with tile.TileContext(nc) as tc:
        with (
            # persistent weights
            tc.tile_pool(name="weights", bufs=1) as wpool,
            # streaming sbuf tiles
            tc.tile_pool(name="xt", bufs=8) as xt_pool,
            tc.tile_pool(name="xt2", bufs=4) as xt2_pool,
            tc.tile_pool(name="featsT", bufs=2) as f_pool,
            tc.tile_pool(name="hT", bufs=2) as h_pool,
            tc.tile_pool(name="p", bufs=SKEW + 2) as p_pool,
            tc.tile_pool(name="f2", bufs=2) as f2_pool,
            tc.tile_pool(name="ot", bufs=4) as o_pool,
            # psum
            tc.tile_pool(name="apsum", bufs=SKEW + 1, space=bass.MemorySpace.PSUM) as a_ps,
            tc.tile_pool(name="f2psum", bufs=2, space=bass.MemorySpace.PSUM) as f2_ps,
            tc.tile_pool(name="smallps", bufs=3, space=bass.MemorySpace.PSUM) as s_ps,
        ):
            # ---- load persistent weights ----
            w_mapT_sb = wpool.tile([M, VC, M], BF16, tag="w_mapT")
            nc.sync.dma_start(
                w_mapT_sb[:], w_mapT_d.rearrange("(c p) m -> p c m", p=M)
            )
            w_map_sb = wpool.tile([M, V], BF16, tag="w_map")
            nc.sync.dma_start(w_map_sb[:], w_map_d[:])
            w1T_sb = wpool.tile([M, K], BF16, tag="w1T")
            nc.sync.dma_start(w1T_sb[:], w1T_d[:])
            b1_sb = wpool.tile([M, KC], F32, tag="b1c")
            nc.sync.dma_start(b1_sb[:], b1c_d[:])
            b2r_sb = wpool.tile([M, M], F32, tag="b2r")
            nc.sync.dma_start(b2r_sb[:], b2r_d[:])

            # W2T resident: (128, KC, M*M) bf16, streamed in 16 column groups
            w2T_sb = wpool.tile([M, KC, M * M], BF16, tag="w2T")
            w2T_ap = w2T_d.rearrange("(c p) q -> p c q", p=M)
            QG = 16  # column-group DMAs
            qg = (M * M) // QG
            for g in range(QG):
                nc.sync.dma_start(
                    w2T_sb[:, :, g * qg : (g + 1) * qg],
                    w2T_ap[:, :, g * qg : (g + 1) * qg],
                )

            ones_sb = wpool.tile([M, 1], BF16, tag="ones")
            nc.vector.memset(ones_sb[:], 1.0)
            ident_sb = wpool.tile([M, M], F32, tag="ident")
            make_identity(nc, ident_sb[:])

            for nb in range(nblk):
                ns = slice(nb * NB, (nb + 1) * NB)

                # ---- s1: featsT = sum_v W_mapT^T xT ----
                feats_psum = s_ps.tile([M, NB], F32, tag="smallps")
                for vc in range(VC):
                    xt = xt_pool.tile([M, NB], BF16, tag="xt")
                    nc.sync.dma_start(xt[:], xTb_d[vc * M : (vc + 1) * M, ns])
                    nc.tensor.matmul(
                        feats_psum[:],
                        w_mapT_sb[:, vc, :],
                        xt[:],
                        start=(vc == 0),
                        stop=(vc == VC - 1),
                    )
                featsT = f_pool.tile([M, NB], BF16, tag="featsT")
                nc.scalar.activation(featsT[:], feats_psum[:], AF.Copy)

                # ---- s2: hT = silu(W1T^T featsT + b1) ----
                hT = h_pool.tile([M, KC, NB], BF16, tag="hT")
                for kc in range(KC):
                    h_psum = s_ps.tile([M, NB], F32, tag="smallps")
                    nc.tensor.matmul(
                        h_psum[:],
                        w1T_sb[:, kc * M : (kc + 1) * M],
                        featsT[:],
                        start=True,
                        stop=True,
                    )
                    if silu_via_sigmoid:
                        # CoreSim has no Silu LUT; emulate z*sigmoid(z)
                        sg = h_pool.tile([M, NB], BF16, tag="sg")
                        nc.scalar.activation(
                            sg[:], h_psum[:], AF.Sigmoid, bias=b1_sb[:, kc : kc + 1]
                        )
                        nc.vector.tensor_mul(hT[:, kc, :], sg[:], h_psum[:])
                    else:
                        nc.scalar.activation(
                            hT[:, kc, :], h_psum[:], AF.Silu, bias=b1_sb[:, kc : kc + 1]
                        )

                # ---- s3 + s4: A_t, P_t, feats2 (software-pipelined) ----
                f2n_psum = f2_ps.tile([M, NCH, M], F32, tag="f2psum")
                p_tiles = {}
                for tt in range(M + SKEW):
                    if tt < M:
                        a_psum = a_ps.tile([M, NB], F32, tag="apsum")
                        for kc in range(KC):
                            nc.tensor.matmul(
                                a_psum[:],
                                w2T_sb[:, kc, tt * M : (tt + 1) * M],
                                hT[:, kc, :],
                                start=(kc == 0),
                                stop=(kc == KC - 1),
                            )
                        p_sb = p_pool.tile([M, NB], BF16, tag="p")
                        # P_t = (A_t + b2[:, t]) * featsT   (fused on DVE)
                        nc.vector.scalar_tensor_tensor(
                            p_sb[:],
                            a_psum[:],
                            b2r_sb[:, tt : tt + 1],
                            featsT[:],
                            op0=ALU.add,
                            op1=ALU.mult,
                        )
                        p_tiles[tt] = p_sb
                    if tt >= SKEW:
                        t0 = tt - SKEW
                        p_prev = p_tiles.pop(t0)
                        for c in range(NCH):
                            nc.tensor.matmul(
                                f2n_psum[:, c, t0 : t0 + 1],
                                p_prev[:, c * M : (c + 1) * M],
                                ones_sb[:],
                                start=True,
                                stop=True,
                            )

                # ---- transpose feats2_nat -> feats2T ----
                f2nat = f2_pool.tile([M, NCH, M], F32, tag="f2nat")
                feats2T = f2_pool.tile([M, NB], BF16, tag="feats2T")
                for c in range(NCH):
                    nc.scalar.activation(f2nat[:, c, :], f2n_psum[:, c, :], AF.Copy)
                    tr_psum = s_ps.tile([M, M], F32, tag="smallps")
                    nc.tensor.transpose(tr_psum[:], f2nat[:, c, :], ident_sb[:])
                    nc.scalar.activation(
                        feats2T[:, c * M : (c + 1) * M], tr_psum[:], AF.Copy
                    )

                # ---- s5: modT = W_map^T feats2T ; out = modT + xT ----
                for vc in range(VC):
                    mod_psum = s_ps.tile([M, NB], F32, tag="smallps")
                    nc.tensor.matmul(
                        mod_psum[:],
                        w_map_sb[:, vc * M : (vc + 1) * M],
                        feats2T[:],
                        start=True,
                        stop=True,
                    )
                    xt2 = xt2_pool.tile([M, NB], F32, tag="xt2")
                    nc.sync.dma_start(xt2[:], xT_d[vc * M : (vc + 1) * M, ns])
                    ot = o_pool.tile([M, NB], F32, tag="ot")
                    nc.vector.tensor_add(ot[:], mod_psum[:], xt2[:])
                    nc.sync.dma_start(out_d[vc * M : (vc + 1) * M, ns], ot[:])

    nc.compile()
    return nc


def make_in_maps(x, W_map, W1, b1, W2, b2, n_cores=N_CORES):
    xf = np.ascontiguousarray(x, dtype=np.float32).reshape(-1, V)
    n_rows = xf.shape[0] // n_cores
    bf = ml_dtypes.bfloat16
    shared = {
        "w_mapT": np.ascontiguousarray(W_map.T.astype(bf)),
        "w_map": np.ascontiguousarray(W_map.astype(bf)),
        "w1T": np.ascontiguousarray(W1.T.astype(bf)),
        "b1c": np.ascontiguousarray(b1.astype(np.float32).reshape(K // M, M).T),
        "w2T": np.ascontiguousarray(W2.T.astype(bf)),
        "b2r": np.ascontiguousarray(b2.astype(np.float32).reshape(M, M).T),
    }
    in_maps = []
    for c in range(n_cores):
        shard = xf[c * n_rows : (c + 1) * n_rows]
        m = dict(shared)
        xT = np.ascontiguousarray(shard.T)
        m["xT"] = xT
        m["xTb"] = xT.astype(bf)
        in_maps.append(m)
    return in_maps


_GRAPH_CACHE = {}


def _get_graph(n_rows):
    if n_rows not in _GRAPH_CACHE:
        _GRAPH_CACHE[n_rows] = build_graph(n_rows)
    return _GRAPH_CACHE[n_rows]


def kernel(x, W_map, W1, b1, W2, b2):
    from concourse.bass_utils import run_bass_kernel_spmd

    pre_shape = x.shape[:-1]
    xf = np.asarray(x, dtype=np.float32).reshape(-1, V)
    n_rows = xf.shape[0] // N_CORES
    nc = _get_graph(n_rows)
    in_maps = make_in_maps(xf, W_map, W1, b1, W2, b2)
    res = run_bass_kernel_spmd(nc, in_maps, core_ids=list(range(N_CORES)))
    outs = [np.asarray(r["out"], dtype=np.float32).T for r in res.results]
    return np.concatenate(outs, axis=0).reshape(*pre_shape, V)


# revision 9
# speedup vs baseline: 1.1048x; 1.1048x over previous
"""AdaptiveVectorModifier Trainium2 kernel (8 NeuronCores, data-parallel over rows).

Reference computation (per row n of x flattened to (N=8192, V=2048)):
    feats = x @ W_map.T                  (N, 128)
    h     = silu(feats @ W1.T + b1)      (N, 512)
    A     = (h @ W2.T + b2)              (N, 128, 128)
    feats2= einsum('nij,nj->ni', A, feats)
    out   = x + feats2 @ W_map

Sharding: rows split 8 ways (1024 rows/core); weights replicated.

Everything on-chip is computed in "transposed space" (rows on the free dim)
so that every matmul contraction dim lands on SBUF partitions:
    s1: featsT (m,n)  = sum_v W_mapT[v,m] xT[v,n]           bf16
    s2: hT     (k,n)  = silu(sum_m W1T[m,k] featsT[m,n]+b1) bf16
    s3: A_t    (j,n)  = sum_k W2T[k, 128t+j] hT[k,n]        bf16 (t = i index)
    s4: P_t    (j,n)  = (A_t + b2[128t+j]) * featsT[j,n]    ACT evac + DVE mul
        feats2_nat[n,t] = sum_j P_t[j,n]                    N=1 matmuls vs ones
    s5: modT   (v,n)  = sum_i W_map[i,v] feats2T[i,n];  outT = modT + xT
Host transposes x per shard on the way in and out.T on the way back.

DMA routing: bulk W2T (16 MiB) streams on the gpsimd SWDGE ring; everything
latency-sensitive (x tiles, weights, outputs) uses the sync HWDGE ring so
the t-loop can start ~10us into the kernel.
"""

import numpy as np
import ml_dtypes

import concourse.bass as bass
import concourse.mybir as mybir
import concourse.tile as tile
from concourse import bacc
from concourse.masks import make_identity

F32 = mybir.dt.float32
BF16 = mybir.dt.bfloat16
AF = mybir.ActivationFunctionType
ALU = mybir.AluOpType

V = 2048     # vector dim
M = 128      # mod dim
K = 512      # hidden (4*M)
NL = 1024    # rows per core
NB = 512     # rows per block
N_CORES = 8
SKEW = 2     # s3 -> reduce software-pipeline skew (t-loop)


def build_graph(n_rows=NL, silu_via_sigmoid=False):
    assert n_rows % NB == 0
    nblk = n_rows // NB

    nc = bacc.Bacc(None, target_bir_lowering=False)

    xT_d = nc.declare_dram_parameter("xT", [V, n_rows], F32, isOutput=False)
    xTb_d = nc.declare_dram_parameter("xTb", [V, n_rows], BF16, isOutput=False)
    w_mapT_d = nc.declare_dram_parameter("w_mapT", [V, M], BF16, isOutput=False)
    w_map_d = nc.declare_dram_parameter("w_map", [M, V], BF16, isOutput=False)
    w1T_d = nc.declare_dram_parameter("w1T", [M, K], BF16, isOutput=False)
    b1c_d = nc.declare_dram_parameter("b1c", [M, K // M], F32, isOutput=False)
    w2T_d = nc.declare_dram_parameter("w2T", [K, M * M], BF16, isOutput=False)
    b2r_d = nc.declare_dram_parameter("b2r", [M, M], F32, isOutput=False)
    out_d = nc.declare_dram_parameter("out", [V, n_rows], F32, isOutput=True)

    VC = V // M            # 16 chunks of the vector dim
    KC = K // M            # 4 chunks of the hidden dim
    NCH = NB // M          # 4 row-chunks per block (for the s4 reduce)

    with tile.TileContext(nc) as tc:
        with (
            tc.tile_pool(name="weights", bufs=1) as wpool,
            tc.tile_pool(name="xt", bufs=8) as xt_pool,
            tc.tile_pool(name="xt2", bufs=4) as xt2_pool,
            tc.tile_pool(name="featsT", bufs=2) as f_pool,
            tc.tile_pool(name="hT", bufs=2) as h_pool,
            tc.tile_pool(name="asb", bufs=SKEW + 1) as a_pool,
            tc.tile_pool(name="p", bufs=SKEW + 2) as p_pool,
            tc.tile_pool(name="f2", bufs=2) as f2_pool,
            tc.tile_pool(name="ot", bufs=4) as o_pool,
            tc.tile_pool(name="apsum", bufs=SKEW + 1, space=bass.MemorySpace.PSUM) as a_ps,
            tc.tile_pool(name="f2psum", bufs=2, space=bass.MemorySpace.PSUM) as f2_ps,
            tc.tile_pool(name="smallps", bufs=3, space=bass.MemorySpace.PSUM) as s_ps,
        ):
            # ---- persistent weights: small ones on the sync ring ----
            w_mapT_sb = wpool.tile([M, VC, M], BF16, tag="w_mapT")
            nc.sync.dma_start(
                w_mapT_sb[:], w_mapT_d.rearrange("(c p) m -> p c m", p=M)
            )
            w_map_sb = wpool.tile([M, V], BF16, tag="w_map")
            nc.sync.dma_start(w_map_sb[:], w_map_d[:])
            w1T_sb = wpool.tile([M, K], BF16, tag="w1T")
            nc.sync.dma_start(w1T_sb[:], w1T_d[:])
            b1_sb = wpool.tile([M, KC], F32, tag="b1c")
            nc.sync.dma_start(b1_sb[:], b1c_d[:])
            b2r_sb = wpool.tile([M, M], F32, tag="b2r")
            nc.sync.dma_start(b2r_sb[:], b2r_d[:])

            ones_sb = wpool.tile([M, 1], BF16, tag="ones")
            nc.vector.memset(ones_sb[:], 1.0)
            ident_sb = wpool.tile([M, M], F32, tag="ident")
            make_identity(nc, ident_sb[:])

            # ---- W2T resident (16 MiB bf16): bulk stream on gpsimd SWDGE ----
            w2T_sb = wpool.tile([M, KC, M * M], BF16, tag="w2T")
            w2T_ap = w2T_d.rearrange("(c p) q -> p c q", p=M)
            QG = 16  # column-group DMAs (1 MiB each), t-group g covers t in [8g, 8g+8)
            qg = (M * M) // QG
            for g in range(QG):
                nc.gpsimd.dma_start(
                    w2T_sb[:, :, g * qg : (g + 1) * qg],
                    w2T_ap[:, :, g * qg : (g + 1) * qg],
                )

            # ---- s1 + s2 for ALL blocks up front (keeps the PE stream dense
            #      across the t-loop boundary between blocks) ----
            featsT = {}
            hT = {}
            for nb in range(nblk):
                ns = slice(nb * NB, (nb + 1) * NB)
                feats_psum = s_ps.tile([M, NB], F32, tag="smallps")
                for vc in range(VC):
                    xt = xt_pool.tile([M, NB], BF16, tag="xt")
                    nc.sync.dma_start(xt[:], xTb_d[vc * M : (vc + 1) * M, ns])
                    nc.tensor.matmul(
                        feats_psum[:],
                        w_mapT_sb[:, vc, :],
                        xt[:],
                        start=(vc == 0),
                        stop=(vc == VC - 1),
                    )
                fT = f_pool.tile([M, NB], BF16, tag="featsT")
                nc.scalar.activation(fT[:], feats_psum[:], AF.Copy)
                featsT[nb] = fT

                hh = h_pool.tile([M, KC, NB], BF16, tag="hT")
                for kc in range(KC):
                    h_psum = s_ps.tile([M, NB], F32, tag="smallps")
                    nc.tensor.matmul(
                        h_psum[:],
                        w1T_sb[:, kc * M : (kc + 1) * M],
                        fT[:],
                        start=True,
                        stop=True,
                    )
                    if silu_via_sigmoid:
                        # CoreSim has no Silu LUT; emulate z*sigmoid(z)
                        sg = h_pool.tile([M, NB], BF16, tag="sg")
                        nc.scalar.activation(
                            sg[:], h_psum[:], AF.Sigmoid, bias=b1_sb[:, kc : kc + 1]
                        )
                        nc.vector.tensor_mul(hh[:, kc, :], sg[:], h_psum[:])
                    else:
                        nc.scalar.activation(
                            hh[:, kc, :], h_psum[:], AF.Silu, bias=b1_sb[:, kc : kc + 1]
                        )
                hT[nb] = hh

            # ---- per block: t-loop (s3+s4), transpose, s5 ----
            for nb in range(nblk):
                ns = slice(nb * NB, (nb + 1) * NB)
                fT = featsT[nb]
                hh = hT[nb]

                f2n_psum = f2_ps.tile([M, NCH, M], F32, tag="f2psum")
                p_tiles = {}
                for tt in range(M + SKEW):
                    if tt < M:
                        a_psum = a_ps.tile([M, NB], F32, tag="apsum")
                        for kc in range(KC):
                            nc.tensor.matmul(
                                a_psum[:],
                                w2T_sb[:, kc, tt * M : (tt + 1) * M],
                                hh[:, kc, :],
                                start=(kc == 0),
                                stop=(kc == KC - 1),
                            )
                        # evac + b2 bias -> bf16 (ScalarE), then * featsT (DVE 2x)
                        a_sb = a_pool.tile([M, NB], BF16, tag="asb")
                        nc.scalar.activation(
                            a_sb[:], a_psum[:], AF.Identity, bias=b2r_sb[:, tt : tt + 1]
                        )
                        p_sb = p_pool.tile([M, NB], BF16, tag="p")
                        nc.vector.tensor_mul(p_sb[:], a_sb[:], fT[:])
                        p_tiles[tt] = p_sb
                    if tt >= SKEW:
                        t0 = tt - SKEW
                        p_prev = p_tiles.pop(t0)
                        for c in range(NCH):
                            nc.tensor.matmul(
                                f2n_psum[:, c, t0 : t0 + 1],
                                p_prev[:, c * M : (c + 1) * M],
                                ones_sb[:],
                                start=True,
                                stop=True,
                            )

                # ---- transpose feats2_nat -> feats2T ----
                f2nat = f2_pool.tile([M, NCH, M], F32, tag="f2nat")
                feats2T = f2_pool.tile([M, NB], BF16, tag="feats2T")
                for c in range(NCH):
                    nc.scalar.activation(f2nat[:, c, :], f2n_psum[:, c, :], AF.Copy)
                    tr_psum = s_ps.tile([M, M], F32, tag="smallps")
                    nc.tensor.transpose(tr_psum[:], f2nat[:, c, :], ident_sb[:])
                    nc.scalar.activation(
                        feats2T[:, c * M : (c + 1) * M], tr_psum[:], AF.Copy
                    )

                # ---- s5: modT = W_map^T feats2T ; out = modT + xT ----
                for vc in range(VC):
                    mod_psum = s_ps.tile([M, NB], F32, tag="smallps")
                    nc.tensor.matmul(
                        mod_psum[:],
                        w_map_sb[:, vc * M : (vc + 1) * M],
                        feats2T[:],
                        start=True,
                        stop=True,
                    )
                    xt2 = xt2_pool.tile([M, NB], F32, tag="xt2")
                    nc.sync.dma_start(xt2[:], xT_d[vc * M : (vc + 1) * M, ns])
                    ot = o_pool.tile([M, NB], F32, tag="ot")
                    nc.vector.tensor_add(ot[:], mod_psum[:], xt2[:])
                    nc.sync.dma_start(out_d[vc * M : (vc + 1) * M, ns], ot[:])

    nc.compile()
    return nc


def make_in_maps(x, W_map, W1, b1, W2, b2, n_cores=N_CORES):
    xf = np.ascontiguousarray(x, dtype=np.float32).reshape(-1, V)
    n_rows = xf.shape[0] // n_cores
    bf = ml_dtypes.bfloat16
    shared = {
        "w_mapT": np.ascontiguousarray(W_map.T.astype(bf)),
        "w_map": np.ascontiguousarray(W_map.astype(bf)),
        "w1T": np.ascontiguousarray(W1.T.astype(bf)),
        "b1c": np.ascontiguousarray(b1.astype(np.float32).reshape(K // M, M).T),
        "w2T": np.ascontiguousarray(W2.T.astype(bf)),
        "b2r": np.ascontiguousarray(b2.astype(np.float32).reshape(M, M).T),
    }
    in_maps = []
    for c in range(n_cores):
        shard = xf[c * n_rows : (c + 1) * n_rows]
        m = dict(shared)
        xT = np.ascontiguousarray(shard.T)
        m["xT"] = xT
        m["xTb"] = xT.astype(bf)
        in_maps.append(m)
    return in_maps


_GRAPH_CACHE = {}


def _get_graph(n_rows):
    if n_rows not in _GRAPH_CACHE:
        _GRAPH_CACHE[n_rows] = build_graph(n_rows)
    return _GRAPH_CACHE[n_rows]


def kernel(x, W_map, W1, b1, W2, b2):
    from concourse.bass_utils import run_bass_kernel_spmd

    pre_shape = x.shape[:-1]
    xf = np.asarray(x, dtype=np.float32).reshape(-1, V)
    n_rows = xf.shape[0] // N_CORES
    nc = _get_graph(n_rows)
    in_maps = make_in_maps(xf, W_map, W1, b1, W2, b2)
    res = run_bass_kernel_spmd(nc, in_maps, core_ids=list(range(N_CORES)))
    outs = [np.asarray(r["out"], dtype=np.float32).T for r in res.results]
    return np.concatenate(outs, axis=0).reshape(*pre_shape, V)


# revision 14
# speedup vs baseline: 1.1437x; 1.0352x over previous
"""AdaptiveVectorModifier Trainium2 kernel (8 NeuronCores, data-parallel over rows).

Reference computation (per row n of x flattened to (N=8192, V=2048)):
    feats = x @ W_map.T                  (N, 128)
    h     = silu(feats @ W1.T + b1)      (N, 512)
    A     = (h @ W2.T + b2)              (N, 128, 128)
    feats2= einsum('nij,nj->ni', A, feats)
    out   = x + feats2 @ W_map

Sharding: rows split 8 ways (1024 rows/core); weights replicated.

Everything on-chip is computed in "transposed space" (rows on the free dim)
so that every matmul contraction dim lands on SBUF partitions:
    s1: featsT (m,n)  = sum_v W_mapT[v,m] xT[v,n]           bf16
    s2: hT     (k,n)  = silu(sum_m W1T[m,k] featsT[m,n]+b1) bf16
    s3: A_t    (j,n)  = sum_k W2T[k, 128t+j] hT[k,n]        bf16 (t = i index)
    s4: P_t    (j,n)  = (A_t + b2[128t+j]) * featsT[j,n]
        feats2_nat[n,t] = sum_j P_t[j,n]                    N=1 matmuls vs ones
    s5: modT   (v,n)  = sum_i W_map[i,v] feats2T[i,n];  outT = modT + xT
Host transposes x per shard on the way in and out.T on the way back.

Scheduling notes (the TensorE stream must stay dense — HAM re-throttles the
PE clock to 1.2 GHz after ~3.4us of idleness):
  - bulk W2T (16 MiB) streams on the gpsimd SWDGE ring; small weights on the
    scalar HWDGE ring; x tiles / outputs split between sync+scalar rings.
  - s1/s2 of block 1 and transpose/s5 of block 0 are interleaved into the
    t-loops so the PE never waits at phase boundaries.
  - stage-4 evac+multiply alternates between DVE (fused scalar_tensor_tensor)
    and ScalarE-evac + DVE-mul by t parity to balance the two engines.
"""

import numpy as np
import ml_dtypes

import concourse.bass as bass
import concourse.mybir as mybir
import concourse.tile as tile
from concourse import bacc
from concourse.masks import make_identity

F32 = mybir.dt.float32
BF16 = mybir.dt.bfloat16
AF = mybir.ActivationFunctionType
ALU = mybir.AluOpType

V = 2048     # vector dim
M = 128      # mod dim
K = 512      # hidden (4*M)
NL = 1024    # rows per core
NB = 512     # rows per block
N_CORES = 8
SKEW = 2     # s3 -> reduce software-pipeline skew (t-loop)


def build_graph(n_rows=NL, silu_via_sigmoid=False):
    assert n_rows % NB == 0
    nblk = n_rows // NB

    nc = bacc.Bacc(None, target_bir_lowering=False)

    xT_d = nc.declare_dram_parameter("xT", [V, n_rows], F32, isOutput=False)
    xTb_d = nc.declare_dram_parameter("xTb", [V, n_rows], BF16, isOutput=False)
    w_mapT_d = nc.declare_dram_parameter("w_mapT", [V, M], BF16, isOutput=False)
    w_map_d = nc.declare_dram_parameter("w_map", [M, V], BF16, isOutput=False)
    w1T_d = nc.declare_dram_parameter("w1T", [M, K], BF16, isOutput=False)
    b1c_d = nc.declare_dram_parameter("b1c", [M, K // M], F32, isOutput=False)
    w2T_d = nc.declare_dram_parameter("w2T", [K, M * M], BF16, isOutput=False)
    b2r_d = nc.declare_dram_parameter("b2r", [M, M], F32, isOutput=False)
    out_d = nc.declare_dram_parameter("out", [V, n_rows], F32, isOutput=True)

    VC = V // M            # 16 chunks of the vector dim
    KC = K // M            # 4 chunks of the hidden dim
    NCH = NB // M          # 4 row-chunks per block (for the s4 reduce)

    with tile.TileContext(nc) as tc:
        with (
            tc.tile_pool(name="weights", bufs=1) as wpool,
            tc.tile_pool(name="xt", bufs=VC + 4) as xt_pool,
            tc.tile_pool(name="xt2", bufs=3) as xt2_pool,
            tc.tile_pool(name="featsT", bufs=2) as f_pool,
            tc.tile_pool(name="hT", bufs=2) as h_pool,
            tc.tile_pool(name="asb", bufs=SKEW + 1) as a_pool,
            tc.tile_pool(name="p", bufs=SKEW + 2) as p_pool,
            tc.tile_pool(name="f2", bufs=2) as f2_pool,
            tc.tile_pool(name="ot", bufs=3) as o_pool,
            tc.tile_pool(name="apsum", bufs=SKEW + 1, space=bass.MemorySpace.PSUM) as a_ps,
            tc.tile_pool(name="f2psum", bufs=2, space=bass.MemorySpace.PSUM) as f2_ps,
            tc.tile_pool(name="smallps", bufs=3, space=bass.MemorySpace.PSUM) as s_ps,
        ):
            # ---- persistent small weights on the scalar HWDGE ring ----
            w_mapT_sb = wpool.tile([M, VC, M], BF16, tag="w_mapT")
            nc.scalar.dma_start(
                w_mapT_sb[:], w_mapT_d.rearrange("(c p) m -> p c m", p=M)
            )
            w_map_sb = wpool.tile([M, V], BF16, tag="w_map")
            nc.scalar.dma_start(w_map_sb[:], w_map_d[:])
            w1T_sb = wpool.tile([M, K], BF16, tag="w1T")
            nc.scalar.dma_start(w1T_sb[:], w1T_d[:])
            b1_sb = wpool.tile([M, KC], F32, tag="b1c")
            nc.scalar.dma_start(b1_sb[:], b1c_d[:])
            b2r_sb = wpool.tile([M, M], F32, tag="b2r")
            nc.scalar.dma_start(b2r_sb[:], b2r_d[:])

            ones_sb = wpool.tile([M, 1], BF16, tag="ones")
            nc.vector.memset(ones_sb[:], 1.0)
            ident_sb = wpool.tile([M, M], F32, tag="ident")
            make_identity(nc, ident_sb[:])

            # ---- W2T resident (16 MiB bf16): bulk stream on gpsimd SWDGE ----
            w2T_sb = wpool.tile([M, KC, M * M], BF16, tag="w2T")
            w2T_ap = w2T_d.rearrange("(c p) q -> p c q", p=M)
            QG = 16  # column-group DMAs (1 MiB each), group g covers t in [8g, 8g+8)
            qg = (M * M) // QG
            for g in range(QG):
                nc.gpsimd.dma_start(
                    w2T_sb[:, :, g * qg : (g + 1) * qg],
                    w2T_ap[:, :, g * qg : (g + 1) * qg],
                )

            # ---- x-tile DMA loads (sync+scalar rings) ----
            xt_tiles = {}

            def emit_xt_loads(nb):
                ns = slice(nb * NB, (nb + 1) * NB)
                for vc in range(VC):
                    xt = xt_pool.tile([M, NB], BF16, tag="xt")
                    eng = nc.sync if vc % 2 == 0 else nc.scalar
                    eng.dma_start(xt[:], xTb_d[vc * M : (vc + 1) * M, ns])
                    xt_tiles[(nb, vc)] = xt

            emit_xt_loads(0)

            featsT = {}
            hT = {}

            def emit_s1_s2(nb):
                feats_psum = s_ps.tile([M, NB], F32, tag="smallps")
                for vc in range(VC):
                    nc.tensor.matmul(
                        feats_psum[:],
                        w_mapT_sb[:, vc, :],
                        xt_tiles.pop((nb, vc))[:],
                        start=(vc == 0),
                        stop=(vc == VC - 1),
                    )
                fT = f_pool.tile([M, NB], BF16, tag="featsT")
                nc.scalar.activation(fT[:], feats_psum[:], AF.Copy)
                featsT[nb] = fT

                hh = h_pool.tile([M, KC, NB], BF16, tag="hT")
                for kc in range(KC):
                    h_psum = s_ps.tile([M, NB], F32, tag="smallps")
                    nc.tensor.matmul(
                        h_psum[:],
                        w1T_sb[:, kc * M : (kc + 1) * M],
                        fT[:],
                        start=True,
                        stop=True,
                    )
                    if silu_via_sigmoid:
                        # CoreSim has no Silu LUT; emulate z*sigmoid(z)
                        sg = h_pool.tile([M, NB], BF16, tag="sg")
                        nc.scalar.activation(
                            sg[:], h_psum[:], AF.Sigmoid, bias=b1_sb[:, kc : kc + 1]
                        )
                        nc.vector.tensor_mul(hh[:, kc, :], sg[:], h_psum[:])
                    else:
                        nc.scalar.activation(
                            hh[:, kc, :], h_psum[:], AF.Silu, bias=b1_sb[:, kc : kc + 1]
                        )
                hT[nb] = hh

            f2n_psums = {}

            def emit_transpose(nb, c):
                # feats2_nat chunk c -> feats2T columns [c*M, (c+1)*M)
                if c == 0:
                    f2n = f2_pool.tile([M, NCH, M], F32, tag="f2nat")
                    f2T = f2_pool.tile([M, NB], BF16, tag="feats2T")
                    emit_transpose.cur = (f2n, f2T)
                f2n, f2T = emit_transpose.cur
                nc.scalar.activation(f2n[:, c, :], f2n_psums[nb][:, c, :], AF.Copy)
                tr_psum = s_ps.tile([M, M], F32, tag="smallps")
                nc.tensor.transpose(tr_psum[:], f2n[:, c, :], ident_sb[:])
                nc.scalar.activation(f2T[:, c * M : (c + 1) * M], tr_psum[:], AF.Copy)
                if c == NCH - 1:
                    feats2T[nb] = f2T

            feats2T = {}

            def emit_s5(nb, vc):
                ns = slice(nb * NB, (nb + 1) * NB)
                mod_psum = s_ps.tile([M, NB], F32, tag="smallps")
                nc.tensor.matmul(
                    mod_psum[:],
                    w_map_sb[:, vc * M : (vc + 1) * M],
                    feats2T[nb][:],
                    start=True,
                    stop=True,
                )
                xt2 = xt2_pool.tile([M, NB], F32, tag="xt2")
                eng = nc.scalar if vc % 2 == 0 else nc.sync
                eng.dma_start(xt2[:], xT_d[vc * M : (vc + 1) * M, ns])
                ot = o_pool.tile([M, NB], F32, tag="ot")
                nc.vector.tensor_add(ot[:], mod_psum[:], xt2[:])
                nc.sync.dma_start(out_d[vc * M : (vc + 1) * M, ns], ot[:])

            def emit_tloop(nb, extra):
                """s3 + s4 software-pipelined t-loop; `extra` maps t -> list of
                emit-closures injected between iterations (deferred work from
                other phases, placed where its inputs are long since ready)."""
                fT = featsT[nb]
                hh = hT[nb]
                f2n_psum = f2_ps.tile([M, NCH, M], F32, tag="f2psum")
                f2n_psums[nb] = f2n_psum
                p_tiles = {}
                for tt in range(M + SKEW):
                    if tt < M:
                        a_psum = a_ps.tile([M, NB], F32, tag="apsum")
                        for kc in range(KC):
                            nc.tensor.matmul(
                                a_psum[:],
                                w2T_sb[:, kc, tt * M : (tt + 1) * M],
                                hh[:, kc, :],
                                start=(kc == 0),
                                stop=(kc == KC - 1),
                            )
                        p_sb = p_pool.tile([M, NB], BF16, tag="p")
                        if tt % 2 == 0:
                            # fused (A + b2) * featsT on DVE (PSUM operand, 1x)
                            nc.vector.scalar_tensor_tensor(
                                p_sb[:],
                                a_psum[:],
                                b2r_sb[:, tt : tt + 1],
                                fT[:],
                                op0=ALU.add,
                                op1=ALU.mult,
                            )
                        else:
                            # ScalarE evac (+b2, ->bf16), then DVE mul at 2x
                            a_sb = a_pool.tile([M, NB], BF16, tag="asb")
                            nc.scalar.activation(
                                a_sb[:], a_psum[:], AF.Identity,
                                bias=b2r_sb[:, tt : tt + 1],
                            )
                            nc.vector.tensor_mul(p_sb[:], a_sb[:], fT[:])
                        p_tiles[tt] = p_sb
                    if tt >= SKEW:
                        t0 = tt - SKEW
                        p_prev = p_tiles.pop(t0)
                        for c in range(NCH):
                            nc.tensor.matmul(
                                f2n_psum[:, c, t0 : t0 + 1],
                                p_prev[:, c * M : (c + 1) * M],
                                ones_sb[:],
                                start=True,
                                stop=True,
                            )
                    for fn in extra.get(tt, ()):
                        fn()

            # ---- emit: s1/s2(0); t-loop(0) with s1/s2(1) injected at t=16;
            #      t-loop(1) with trans(0) at t=0..3 and s5(0) spread t=8..40;
            #      then trans(1) + s5(1) ----
            emit_s1_s2(0)
            if nblk == 1:
                emit_tloop(0, {})
                for c in range(NCH):
                    emit_transpose(0, c)
                for vc in range(VC):
                    emit_s5(0, vc)
            else:
                assert nblk == 2
                emit_tloop(
                    0, {0: [lambda: emit_xt_loads(1)], 16: [lambda: emit_s1_s2(1)]}
                )
                extra = {c: [lambda c=c: emit_transpose(0, c)] for c in range(NCH)}
                for vc in range(VC):
                    extra.setdefault(8 + 2 * vc, []).append(
                        lambda vc=vc: emit_s5(0, vc)
                    )
                emit_tloop(1, extra)
                for c in range(NCH):
                    emit_transpose(1, c)
                for vc in range(VC):
                    emit_s5(1, vc)

    nc.compile()
    return nc


def make_in_maps(x, W_map, W1, b1, W2, b2, n_cores=N_CORES):
    xf = np.ascontiguousarray(x, dtype=np.float32).reshape(-1, V)
    n_rows = xf.shape[0] // n_cores
    bf = ml_dtypes.bfloat16
    shared = {
        "w_mapT": np.ascontiguousarray(W_map.T.astype(bf)),
        "w_map": np.ascontiguousarray(W_map.astype(bf)),
        "w1T": np.ascontiguousarray(W1.T.astype(bf)),
        "b1c": np.ascontiguousarray(b1.astype(np.float32).reshape(K // M, M).T),
        "w2T": np.ascontiguousarray(W2.T.astype(bf)),
        "b2r": np.ascontiguousarray(b2.astype(np.float32).reshape(M, M).T),
    }
    in_maps = []
    for c in range(n_cores):
        shard = xf[c * n_rows : (c + 1) * n_rows]
        m = dict(shared)
        xT = np.ascontiguousarray(shard.T)
        m["xT"] = xT
        m["xTb"] = xT.astype(bf)
        in_maps.append(m)
    return in_maps


_GRAPH_CACHE = {}


def _get_graph(n_rows):
    if n_rows not in _GRAPH_CACHE:
        _GRAPH_CACHE[n_rows] = build_graph(n_rows)
    return _GRAPH_CACHE[n_rows]


def kernel(x, W_map, W1, b1, W2, b2):
    from concourse.bass_utils import run_bass_kernel_spmd

    pre_shape = x.shape[:-1]
    xf = np.asarray(x, dtype=np.float32).reshape(-1, V)
    n_rows = xf.shape[0] // N_CORES
    nc = _get_graph(n_rows)
    in_maps = make_in_maps(xf, W_map, W1, b1, W2, b2)
    res = run_bass_kernel_spmd(nc, in_maps, core_ids=list(range(N_CORES)))
    outs = [np.asarray(r["out"], dtype=np.float32).T for r in res.results]
    return np.concatenate(outs, axis=0).reshape(*pre_shape, V)
